# revision 1
# baseline (speedup 1.0000x reference)
"""CSPN 3x3 propagation step on 8 Trainium2 NeuronCores.

out[b,0,r,c] = sum_k aff[b,k,r,c] * patch_k(cur)[r,c], with the center tap
(k=4) taken from coarse_seg instead of cur_seg. Zero padding at image edges.

Sharding: pure data parallel over batch (16 images -> 2 per core), one SPMD
Bass program run on all 8 cores with per-core input slices.

Per-core algorithm (per 512x512 image): rows are packed PARTITION-MAJOR,
r = 4p + t  (partition p in 0..127, sub-row t in 0..3), so a +-1 row shift
stays inside the partition (a free-dim offset) for 3 of the 4 sub-rows.
The block-edge rows (r = 4p-1 and r = 4p+4) are covered by two small
[128, 512] edge-plane loads (stride-4 row gather from HBM, 256 KB each).

Division of labor (keeps the kernel purely DMA-ring-bound):
  - The two HWDGE rings carry everything with 8 KB descriptors, balanced
    at 6.55 MB per ring per image, critical tiles first:
      ACT: a7, tM[0:2], tU, a8, a1, a3, tC, out[0:2]
      SP:  a6, tM[2:4], a0, tD, a2, a5, a4, out[2:4]
  - DVE and Pool (GpSimd) compute ONLY the 9 elementwise products
    P_k = aff_k * shifted cur (dx = free-dim column offset into
    zero-padded tiles; dy = sub-row offset or edge plane; k=4 multiplies
    coarse_seg instead).
  - The TensorEngine sums the first 8 products in PSUM via identity
    matmuls in float32r mode (1 cycle/row; exact 0/1 weights, inputs
    rounded to ~tf32 — orders of magnitude inside the 2e-2 gate).
  - The root: out_t = psum_t + m4 (the product of the two LAST-arriving
    tiles, a4 and tC) — one mul and two bank adds per engine after the
    final DMA lands, then the halves store on their rings.
  - Zero-pad regions (columns 0/513, edge-plane boundary rows) live in
    PERSISTENT double-buffered tiles memset once before the image loop, so
    no per-image memsets gate the DMA streams.
"""

import sys

import numpy as np

if "/opt/trn_rl_repo" not in sys.path:
    sys.path.insert(0, "/opt/trn_rl_repo")

B_PER_CORE = 2
N_CORES = 8
H = 512
W = 512
NBLK = 4  # sub-rows per partition
WPAD = W + 2  # zero column on each side

_compiled = None
_compiled_reps = {}


def _build_program(reps=1):
    """reps>1 unrolls the whole per-core computation `reps` times inside one
    NEFF — used only to measure kernel time through the dispatch noise."""
    import concourse.bacc as bacc
    import concourse.mybir as mybir
    import concourse.tile as tile

    fp32 = mybir.dt.float32
    fp32r = mybir.dt.float32r

    nc = bacc.Bacc(
        "TRN2",
        target_bir_lowering=False,
        debug=False,
        enable_asserts=False,
        num_devices=N_CORES,
    )

    aff_d = nc.dram_tensor(
        "affinity", [B_PER_CORE, 9, H, W], fp32, kind="ExternalInput"
    ).ap()
    cur_d = nc.dram_tensor(
        "cur_seg", [B_PER_CORE, 1, H, W], fp32, kind="ExternalInput"
    ).ap()
    coa_d = nc.dram_tensor(
        "coarse_seg", [B_PER_CORE, 1, H, W], fp32, kind="ExternalInput"
    ).ap()
    idn_d = nc.dram_tensor("ident", [128, 128], fp32r, kind="ExternalInput").ap()
    out_d = nc.dram_tensor(
        "out", [B_PER_CORE, 1, H, W], fp32, kind="ExternalOutput"
    ).ap()

    with tile.TileContext(nc) as tc:
        with (
            tc.tile_pool(name="idn", bufs=1) as idn_pool,
            tc.tile_pool(name="cur", bufs=2) as cur_pool,
            tc.tile_pool(name="edge", bufs=4) as edge_pool,
            tc.tile_pool(name="coa", bufs=2) as coa_pool,
            tc.tile_pool(name="aff", bufs=7) as aff_pool,
            tc.tile_pool(name="asw", bufs=2) as asw_pool,
            tc.tile_pool(name="prod", bufs=4) as prod_pool,
            tc.tile_pool(name="half", bufs=4) as half_pool,
            tc.tile_pool(name="ps", bufs=4, space="PSUM") as ps_pool,
        ):
            tI = idn_pool.tile([128, 128], fp32r)
            nc.scalar.dma_start(out=tI[:], in_=idn_d[:])

            # Persistent double-buffered tiles: pad regions zeroed ONCE here;
            # per-image DMAs only overwrite the data regions.
            tM_bufs, tU_bufs, tD_bufs = [], [], []
            for par in range(2):
                tM = cur_pool.tile([128, NBLK, WPAD], fp32, name=f"tM{par}")
                nc.vector.memset(tM[:, :, 0:1], 0.0)
                nc.vector.memset(tM[:, :, WPAD - 1 : WPAD], 0.0)
                tM_bufs.append(tM)
                tU = edge_pool.tile([128, WPAD], fp32, name=f"tU{par}")
                nc.gpsimd.memset(tU[:], 0.0)
                tU_bufs.append(tU)
                tD = edge_pool.tile([128, WPAD], fp32, name=f"tD{par}")
                nc.gpsimd.memset(tD[:], 0.0)
                tD_bufs.append(tD)

            for i, b in enumerate(
                [bb for _ in range(reps) for bb in range(B_PER_CORE)]
            ):
                par = i % 2

                def _load_aff(k, ring):
                    ak = aff_pool.tile([128, NBLK, W], fp32, tag="aff")
                    ring.dma_start(
                        out=ak[:],
                        in_=aff_d[b, k].rearrange("(p t) c -> p t c", p=128),
                    )
                    return ak

                # PSUM accumulators: two 2-bank tiles (sub-rows 0:2 / 2:4)
                ps_lo = ps_pool.tile([128, 2, W], fp32, tag="ps", name=f"pslo{par}")
                ps_hi = ps_pool.tile([128, 2, W], fp32, tag="ps", name=f"pshi{par}")
                started = [False] * NBLK

                def _accum(P, last=False):
                    """psum_t += I.T @ P[:, t, :]  (f32r, 1 cycle/row)."""
                    for t in range(NBLK):
                        pst = ps_lo[:, t, :] if t < 2 else ps_hi[:, t - 2, :]
                        nc.tensor.matmul(
                            pst,
                            tI[:],
                            P[:, t, :],
                            start=not started[t],
                            stop=last,
                        )
                        started[t] = True

                # --- DMA streams (program order == intended ring order) ---
                # a1 and a2 ride the Pool software-DGE: each blocks the Pool
                # engine ~4.2 us, slotted into Pool's idle window at image
                # start — a third DMA channel that takes 6.3 us/image off the
                # two HWDGE rings (now 5.5 MB = 16.7 us each per image).
                a7 = _load_aff(7, nc.scalar)
                a6 = _load_aff(6, nc.sync)
                a1 = asw_pool.tile([128, NBLK, W], fp32, tag="asw")
                nc.gpsimd.dma_start(
                    out=a1[:], in_=aff_d[b, 1].rearrange("(p t) c -> p t c", p=128)
                )

                tM = tM_bufs[par]
                cur_rows = cur_d[b, 0].rearrange("(p t) c -> p t c", p=128)
                nc.scalar.dma_start(out=tM[:, 0:2, 1 : W + 1], in_=cur_rows[:, 0:2, :])
                nc.sync.dma_start(out=tM[:, 2:4, 1 : W + 1], in_=cur_rows[:, 2:4, :])
                a2 = asw_pool.tile([128, NBLK, W], fp32, tag="asw")
                nc.gpsimd.dma_start(
                    out=a2[:], in_=aff_d[b, 2].rearrange("(p t) c -> p t c", p=128)
                )

                tU = tU_bufs[par]
                up_rows = cur_d[b, 0][4:H].rearrange("(p t) c -> p t c", t=4)
                nc.scalar.dma_start(out=tU[0:127, 1 : W + 1], in_=up_rows[:, 0, :])
                a0 = _load_aff(0, nc.sync)

                a8 = _load_aff(8, nc.scalar)
                tD = tD_bufs[par]
                dn_rows = cur_d[b, 0][3 : H - 1].rearrange("(p t) c -> p t c", t=4)
                nc.sync.dma_start(out=tD[1:128, 1 : W + 1], in_=dn_rows[:, 0, :])

                a3 = _load_aff(3, nc.scalar)
                a5 = _load_aff(5, nc.sync)
                tC = coa_pool.tile([128, NBLK, W], fp32, tag="coa")
                nc.scalar.dma_start(
                    out=tC[:], in_=coa_d[b, 0].rearrange("(p t) c -> p t c", p=128)
                )
                a4 = _load_aff(4, nc.sync)

                # dx column windows into the padded tiles
                def mwin(tlo, thi, dxi):
                    return tM[:, tlo:thi, dxi : dxi + W]

                # --- products in arrival order; PE accumulates each.
                # Pool is busy with the swdge transfers early, so the first
                # products run on DVE; Pool takes the later-arriving planes.
                P6 = prod_pool.tile([128, NBLK, W], fp32r, tag="prod")
                nc.vector.tensor_mul(out=P6[:, 0:3, :], in0=a6[:, 0:3, :], in1=mwin(1, 4, 0))
                nc.vector.tensor_mul(out=P6[:, 3, :], in0=a6[:, 3, :], in1=tU[:, 0:W])
                _accum(P6)
                P7 = prod_pool.tile([128, NBLK, W], fp32r, tag="prod")
                nc.vector.tensor_mul(out=P7[:, 0:3, :], in0=a7[:, 0:3, :], in1=mwin(1, 4, 1))
                nc.vector.tensor_mul(out=P7[:, 3, :], in0=a7[:, 3, :], in1=tU[:, 1 : 1 + W])
                _accum(P7)

                P1 = prod_pool.tile([128, NBLK, W], fp32r, tag="prod")
                nc.vector.tensor_mul(out=P1[:, 1:4, :], in0=a1[:, 1:4, :], in1=mwin(0, 3, 1))
                nc.vector.tensor_mul(out=P1[:, 0, :], in0=a1[:, 0, :], in1=tD[:, 1 : 1 + W])
                _accum(P1)
                P0 = prod_pool.tile([128, NBLK, W], fp32r, tag="prod")
                nc.vector.tensor_mul(out=P0[:, 1:4, :], in0=a0[:, 1:4, :], in1=mwin(0, 3, 0))
                nc.vector.tensor_mul(out=P0[:, 0, :], in0=a0[:, 0, :], in1=tD[:, 0:W])
                _accum(P0)

                P8 = prod_pool.tile([128, NBLK, W], fp32r, tag="prod")
                nc.gpsimd.tensor_mul(out=P8[:, 0:3, :], in0=a8[:, 0:3, :], in1=mwin(1, 4, 2))
                nc.gpsimd.tensor_mul(out=P8[:, 3, :], in0=a8[:, 3, :], in1=tU[:, 2 : 2 + W])
                _accum(P8)
                P2 = prod_pool.tile([128, NBLK, W], fp32r, tag="prod")
                nc.vector.tensor_mul(out=P2[:, 1:4, :], in0=a2[:, 1:4, :], in1=mwin(0, 3, 2))
                nc.vector.tensor_mul(out=P2[:, 0, :], in0=a2[:, 0, :], in1=tD[:, 2 : 2 + W])
                _accum(P2)

                # dy=0 side tap k=5 is the last PE-accumulated plane
                M5 = prod_pool.tile([128, NBLK, W], fp32r, tag="prod")
                nc.gpsimd.tensor_mul(out=M5[:], in0=a5[:], in1=mwin(0, 4, 2))
                _accum(M5, last=True)

                # --- root: out = (ps7 + a3*cur) + a4*coarse, half-granular.
                # k=3 and k=4 fold at the root with one mul + one add each,
                # so whichever of a3/tC/a4 the scheduler lands last, the
                # post-arrival chain stays two adds deep.
                out_rows = out_d[b, 0].rearrange("(p t) c -> p t c", p=128)
                Osb = prod_pool.tile([128, NBLK, W], fp32, tag="prod")
                m3lo = half_pool.tile([128, 2, W], fp32, tag="half")
                nc.gpsimd.tensor_mul(
                    out=m3lo[:], in0=a3[:, 0:2, :], in1=tM[:, 0:2, 0:W]
                )
                m3hi = half_pool.tile([128, 2, W], fp32, tag="half")
                nc.gpsimd.tensor_mul(
                    out=m3hi[:], in0=a3[:, 2:4, :], in1=tM[:, 2:4, 0:W]
                )
                m4lo = half_pool.tile([128, 2, W], fp32, tag="half")
                nc.gpsimd.tensor_mul(out=m4lo[:], in0=a4[:, 0:2, :], in1=tC[:, 0:2, :])
                m4hi = half_pool.tile([128, 2, W], fp32, tag="half")
                nc.gpsimd.tensor_mul(out=m4hi[:], in0=a4[:, 2:4, :], in1=tC[:, 2:4, :])
                nc.vector.tensor_add(out=Osb[:, 0, :], in0=ps_lo[:, 0, :], in1=m3lo[:, 0, :])
                nc.vector.tensor_add(out=Osb[:, 1, :], in0=ps_lo[:, 1, :], in1=m3lo[:, 1, :])
                nc.gpsimd.tensor_add(out=Osb[:, 0, :], in0=Osb[:, 0, :], in1=m4lo[:, 0, :])
                nc.scalar.dma_start(out=out_rows[:, 0, :], in_=Osb[:, 0, :])
                nc.gpsimd.tensor_add(out=Osb[:, 1, :], in0=Osb[:, 1, :], in1=m4lo[:, 1, :])
                nc.scalar.dma_start(out=out_rows[:, 1, :], in_=Osb[:, 1, :])
                nc.vector.tensor_add(out=Osb[:, 2, :], in0=ps_hi[:, 0, :], in1=m3hi[:, 0, :])
                nc.vector.tensor_add(out=Osb[:, 3, :], in0=ps_hi[:, 1, :], in1=m3hi[:, 1, :])
                nc.gpsimd.tensor_add(out=Osb[:, 2, :], in0=Osb[:, 2, :], in1=m4hi[:, 0, :])
                nc.sync.dma_start(out=out_rows[:, 2, :], in_=Osb[:, 2, :])
                nc.gpsimd.tensor_add(out=Osb[:, 3, :], in0=Osb[:, 3, :], in1=m4hi[:, 1, :])
                nc.sync.dma_start(out=out_rows[:, 3, :], in_=Osb[:, 3, :])

    nc.compile()
    return nc


def _get_program(reps=1):
    global _compiled
    if reps != 1:
        if reps not in _compiled_reps:
            _compiled_reps[reps] = _build_program(reps)
        return _compiled_reps[reps]
    if _compiled is None:
        _compiled = _build_program()
    return _compiled


def _in_maps(affinity, cur_seg, coarse_seg):
    ident = np.eye(128, dtype=np.float32)
    maps = []
    for j in range(N_CORES):
        s = slice(j * B_PER_CORE, (j + 1) * B_PER_CORE)
        maps.append(
            {
                "affinity": np.ascontiguousarray(affinity[s]),
                "cur_seg": np.ascontiguousarray(cur_seg[s]),
                "coarse_seg": np.ascontiguousarray(coarse_seg[s]),
                "ident": ident,
            }
        )
    return maps


def kernel(affinity, cur_seg, coarse_seg, i=None, **_unused):
    from concourse.bass_utils import run_bass_kernel_spmd

    nc = _get_program()

    affinity = np.ascontiguousarray(affinity, dtype=np.float32)
    cur_seg = np.ascontiguousarray(cur_seg, dtype=np.float32)
    coarse_seg = np.ascontiguousarray(coarse_seg, dtype=np.float32)

    res = run_bass_kernel_spmd(
        nc, _in_maps(affinity, cur_seg, coarse_seg), core_ids=list(range(N_CORES))
    )
    out = np.concatenate([r["out"] for r in res.results], axis=0)
    return out

